# revision 27
# baseline (speedup 1.0000x reference)
"""Trainium2 Bass kernel for a binarized 4-layer MLP (dense_mlp).

Net (per reference):
  h = sign(x) @ sign(w1).T + b1 ; h = clip(bn1(h), -1, 1)
  h = sign(h) @ sign(w2).T + b2 ; h = clip(bn2(h), -1, 1)
  h = sign(h) @ sign(w3).T + b3 ; h = clip(bn3(h), -1, 1)
  logits = h @ w4.T + b4 ; out = log_softmax(logits)   # 2 classes

Strategy: pure data parallel over 8 cores (batch 131072 -> 8 x 16384).
Host prep: x transposed to feature-major bf16 [81, B] and sharded by
columns; sign/BN/bias/log-softmax algebra folded into device weights +
thresholds.

v2 design (vs v1 baseline at ~194us):
  - compound matmuls: one matmul instruction per (weight, 2048-col group)
    spanning 4 PSUM banks -> 1 LDWEIGHTS per weight use instead of 1 per
    512-col matmul (LDWEIGHTS serialized ~70us of PE time in v1).
  - head: dw folded into L3's affine (scale=s3*dw, bias=e3*dw) and clamp
    (+-|dw|); the 3 feature planes are pre-summed (DVE 2x / GpSimd) so the
    head matmul is a single ones-vector reduction; its 4 512-col chunks
    land on PSUM partitions {0,32,64,96} so the drain is one [4,512] copy.
  - x shipped as bf16 (sign-preserving), u0 compare on GpSimd.
  - L3 clamp on DVE in 4x perf mode (all-bf16 SBUF operands).
  - binarize/affine planes split between ACT and DVE by a tunable table.
"""

import os
import sys

import numpy as np

for _p in ("/opt/trn_rl_repo", "/root/.axon_site/_ro/trn_rl_repo"):
    if os.path.isdir(_p) and _p not in sys.path:
        sys.path.insert(0, _p)

import ml_dtypes  # noqa: E402

BF16 = ml_dtypes.bfloat16
FP8 = ml_dtypes.float8_e4m3

# Problem constants (hardcoded per contract)
B_FULL = 131072
N_CORES = 8
NB = B_FULL // N_CORES  # 16384 rows per core
IN = 81
H = 384
EPS = 1e-5
P = 128
CG = 2048            # columns per group (compound matmul free dim)
NCG = NB // CG       # 8 groups

# Engine assignment tables ("A" = ACT/scalar, "D" = DVE/vector).
# Binarize planes must be whole-plane (form is baked into next layer's
# weights): "A" -> s-form (+-1 via Sign), "D" -> u-form ({0,1} via is_gt).
BIN1 = ["A", "A", "A"]  # L1 output planes m=0,1,2
BIN2 = ["A", "A", "A"]  # L2 output planes
# L3 affine is form-free; assignable per (plane, colgroup).
# ~2 of 24 blobs on ACT (measured: ACT 1.87us/blob vs DVE 2.4; ACT carries
# all six binarize planes so most affines go to DVE).
AFF = [
    ["D"] * NCG,                                    # m=0
    ["D"] * NCG,                                    # m=1
    (["A", "D", "D", "D"] * ((NCG + 3) // 4))[:NCG],  # m=2
]

_CACHE = {}


def _build_program():
    import concourse.bacc as bacc
    import concourse.bass as bass  # noqa: F401
    import concourse.tile as tile
    from concourse import mybir

    f32 = mybir.dt.float32
    bf16 = mybir.dt.bfloat16
    fp8 = mybir.dt.float8e4
    DR = mybir.MatmulPerfMode.DoubleRow
    AF = mybir.ActivationFunctionType
    ALU = mybir.AluOpType

    nc = bacc.Bacc("TRN2", target_bir_lowering=False, debug=False)

    xt_d = nc.dram_tensor("xt", [IN, NB], bf16, kind="ExternalInput").ap()
    w1t_d = nc.dram_tensor("w1t", [IN, H], bf16, kind="ExternalInput").ap()
    w2t_d = nc.dram_tensor("w2t", [P, 1536], fp8, kind="ExternalInput").ap()
    w3t_d = nc.dram_tensor("w3t", [P, 1536], fp8, kind="ExternalInput").ap()
    ones_d = nc.dram_tensor("ones", [P, 1], bf16, kind="ExternalInput").ap()
    aux_d = nc.dram_tensor("aux", [P, 28], f32, kind="ExternalInput").ap()
    out_d = nc.dram_tensor("out", [NB, 2], f32, kind="ExternalOutput").ap()

    # aux columns:
    # 0-2: phi1[m] (u-form is_gt threshold), 3-5: -phi1[m] (s-form Sign bias)
    # 6-8: phi2[m], 9-11: -phi2[m]
    # 12-14: scale3[m] = s3*dw, 15-17: bias3[m] = dw*(c3 - s3*corr3)
    # 18-20: +|dw|[m], 21-23: -|dw|[m]
    # 24: db, 25: -db
    with tile.TileContext(nc) as tc:
        with (
            tc.tile_pool(name="consts", bufs=1) as cpool,
            tc.tile_pool(name="xin", bufs=3) as xpool,
            tc.tile_pool(name="u0", bufs=3) as u0pool,
            tc.tile_pool(name="u1", bufs=2) as u1pool,
            tc.tile_pool(name="u2", bufs=2) as u2pool,
            tc.tile_pool(name="taff", bufs=4) as tpool,
            tc.tile_pool(name="hcl", bufs=8) as hpool,
            tc.tile_pool(name="dsb", bufs=2) as dsbpool,
            tc.tile_pool(name="fin", bufs=1) as fpool,
            tc.tile_pool(name="mm", bufs=8 // (CG // 512), space="PSUM") as pspool,
            tc.tile_pool(name="dram", bufs=1, space="DRAM") as dpool,
        ):
            dscr = dpool.tile([NB], f32)
            # ---- constants ----
            w1s = cpool.tile([IN, H], bf16)
            nc.sync.dma_start(w1s[:], w1t_d[:])
            w2s = cpool.tile([P, 1536], fp8)
            nc.sync.dma_start(w2s[:], w2t_d[:])
            w3s = cpool.tile([P, 1536], fp8)
            nc.sync.dma_start(w3s[:], w3t_d[:])
            w2r = w2s.rearrange("p (s i c) -> p s i c", i=2, c=P)
            w3r = w3s.rearrange("p (s i c) -> p s i c", i=2, c=P)
            ones = cpool.tile([P, 1], bf16)
            nc.sync.dma_start(ones[:], ones_d[:])
            aux = cpool.tile([P, 28], f32)
            nc.sync.dma_start(aux[:], aux_d[:])

            def emit_u0(g):
                xf = xpool.tile([IN, CG], bf16, tag="xf", name=f"xf_{g}")
                nc.sync.dma_start(xf[:], xt_d[:, g * CG : (g + 1) * CG])
                u0 = u0pool.tile([IN, CG], bf16, tag="u0", name=f"u0_{g}")
                nc.vector.tensor_scalar(u0[:], xf[:], 0.0, None, ALU.is_gt)
                return u0

            u0 = emit_u0(0)
            for g in range(NCG):
                # prefetch next colgroup's input compare so the PE has L1
                # work available during this colgroup's head tail
                u0_next = emit_u0(g + 1) if g + 1 < NCG else None

                # ---- L1 (bf16, K=81; weights loaded once per 4 chunks) ----
                u1 = u1pool.tile([P, 3, CG], fp8, tag="u1", name=f"u1_{g}")
                for m in range(3):
                    ps = pspool.tile([P, CG], f32, tag="ps", name=f"ps1_{g}_{m}")
                    for c in range(4):
                        nc.tensor.matmul(
                            ps[:, 512 * c : 512 * (c + 1)],
                            w1s[:, m * P : (m + 1) * P],
                            u0[:, 512 * c : 512 * (c + 1)],
                            start=True, stop=True,
                        )
                    if BIN1[m] == "A":
                        nc.scalar.activation(
                            u1[:, m, :], ps[:], AF.Sign,
                            bias=aux[:, 3 + m : 4 + m], scale=1.0,
                        )
                    else:
                        nc.vector.tensor_scalar(
                            u1[:, m, :], ps[:], aux[:, m : m + 1], None,
                            ALU.is_gt,
                        )

                # ---- L2 (fp8 DoubleRow, K=512 padded, compound) ----
                u2 = u2pool.tile([P, 3, CG], fp8, tag="u2", name=f"u2_{g}")
                for m in range(3):
                    ps = pspool.tile([P, CG], f32, tag="ps", name=f"ps2_{g}_{m}")
                    for kh in range(2):
                        for c in range(4):
                            nc.tensor.matmul(
                                ps[:, 512 * c : 512 * (c + 1)],
                                w2r[:, kh * 3 + m, :, :],
                                u1[:, kh : kh + 2, 512 * c : 512 * (c + 1)],
                                start=(kh == 0), stop=(kh == 1),
                                perf_mode=DR,
                            )
                    if BIN2[m] == "A":
                        nc.scalar.activation(
                            u2[:, m, :], ps[:], AF.Sign,
                            bias=aux[:, 9 + m : 10 + m], scale=1.0,
                        )
                    else:
                        nc.vector.tensor_scalar(
                            u2[:, m, :], ps[:], aux[:, 6 + m : 7 + m], None,
                            ALU.is_gt,
                        )

                # ---- L3 (fp8 DoubleRow) + affine(+dw) + clamp(+-|dw|) ----
                # clamp/pre-sum run at half-colgroup granularity to shorten
                # the serial tail; h01 halves go to GpSimd (its Add path is
                # well-behaved), hsum stays on DVE.
                hcl = []
                ps3 = []
                HH = CG // 2
                for m in range(3):
                    ps = pspool.tile([P, CG], f32, tag="ps", name=f"ps3_{g}_{m}")
                    ps3.append(ps)
                    for kh in range(2):
                        for c in range(4):
                            nc.tensor.matmul(
                                ps[:, 512 * c : 512 * (c + 1)],
                                w3r[:, kh * 3 + m, :, :],
                                u2[:, kh : kh + 2, 512 * c : 512 * (c + 1)],
                                start=(kh == 0), stop=(kh == 1),
                                perf_mode=DR,
                            )
                    t = tpool.tile([P, CG], bf16, tag="t3", name=f"t3_{g}_{m}")
                    if AFF[m][g] == "A":
                        nc.scalar.activation(
                            t[:], ps[:], AF.Identity,
                            bias=aux[:, 15 + m : 16 + m],
                            scale=aux[:, 12 + m : 13 + m],
                        )
                    else:
                        nc.vector.tensor_scalar(
                            t[:], ps[:],
                            aux[:, 12 + m : 13 + m], aux[:, 15 + m : 16 + m],
                            ALU.mult, ALU.add,
                        )
                    # clamp to +-|dw| -> bf16 (DVE 4x mode), in halves
                    h = hpool.tile([P, CG], bf16, tag="h3", name=f"h3_{g}_{m}")
                    for hh in range(2):
                        sl = slice(hh * HH, (hh + 1) * HH)
                        nc.vector.tensor_scalar(
                            h[:, sl], t[:, sl],
                            aux[:, 18 + m : 19 + m], aux[:, 21 + m : 22 + m],
                            ALU.min, ALU.max,
                        )
                    hcl.append(h)
                    if m == 1:
                        h01 = hpool.tile([P, CG], bf16, tag="h01",
                                         name=f"h01_{g}")
                        for hh in range(2):
                            sl = slice(hh * HH, (hh + 1) * HH)
                            nc.vector.tensor_tensor(
                                h01[:, sl], hcl[0][:, sl], hcl[1][:, sl],
                                ALU.add,
                            )

                hsum = hpool.tile([P, CG], bf16, tag="hsum", name=f"hsum_{g}")
                psh = ps3[2]  # head reuses the last L3 tile (freed latest)
                NCH = CG // 512  # head chunks per colgroup
                for hh in range(2):
                    sl = slice(hh * HH, (hh + 1) * HH)
                    nc.vector.tensor_tensor(
                        hsum[:, sl], h01[:, sl], hcl[2][:, sl], ALU.add
                    )
                    # head: ones-reduction; chunk c -> partition 32c
                    for c in range(hh * NCH // 2, (hh + 1) * NCH // 2):
                        nc.tensor.matmul(
                            psh[32 * c : 32 * c + 1, 0:512],
                            ones[:],
                            hsum[:, 512 * c : 512 * (c + 1)],
                            start=True, stop=True,
                            skip_group_check=True,
                            tile_position=(0, 32 * c),
                        )
                # drain: engines can't stride partitions, but copying the
                # whole partition range costs only the 512 free-dim cycles;
                # the DMA then gathers rows {0,32,..} from SBUF.
                nrow = 32 * (NCH - 1) + 1
                dsbg = dsbpool.tile([P, 512], f32, tag="dsb", name=f"dsb_{g}")
                nc.vector.tensor_copy(dsbg[0:nrow, :], psh[0:nrow, 0:512])
                dsbv = dsbg.rearrange("(a b) f -> a b f", b=32)
                nc.sync.dma_start(
                    dscr[g * CG : (g + 1) * CG].rearrange(
                        "(r one f) -> r one f", one=1, f=512
                    ),
                    dsbv[0:NCH, 0:1, :],
                )
                u0 = u0_next

            # ---- tail: respread via DRAM, softplus pair, interleave ----
            d2 = fpool.tile([P, P], f32, tag="d2")
            nc.sync.dma_start(d2[:], dscr.rearrange("(p j) -> p j", j=P))
            sneg = fpool.tile([P, P], f32, tag="sneg")
            nc.scalar.activation(
                sneg[:], d2[:], AF.Sigmoid, bias=aux[:, 25:26], scale=-1.0
            )
            spos = fpool.tile([P, P], f32, tag="spos")
            nc.scalar.activation(
                spos[:], d2[:], AF.Sigmoid, bias=aux[:, 24:25], scale=1.0
            )
            out_t = fpool.tile([P, 2 * P], f32, tag="outt")
            ov = out_t.rearrange("p (j c) -> p j c", c=2)
            nc.scalar.activation(ov[:, :, 0], sneg[:], AF.Ln)
            nc.scalar.activation(ov[:, :, 1], spos[:], AF.Ln)
            nc.sync.dma_start(
                out_d.rearrange("(p j) c -> p (j c)", j=P), out_t[:]
            )

    nc.compile()
    if os.environ.get('KDEDUP', '1') == '1':
        _dedupe_ldweights(nc)
    return nc


def _dedupe_ldweights(nc):
    """Remove InstLdweights whose weights AP matches the previous load on
    the PE stream (the PE array keeps weights until the next load). Only
    instructions with no semaphore waits/updates are removed; anything
    synced acts as a barrier and refreshes the tracked state."""
    total = 0
    for f in nc.m.functions:
        for bb in f.blocks:
            insts = bb.instructions
            keep = []
            last_key = None
            removed = 0
            for inst in insts:
                nm = type(inst).__name__
                if nm == "InstLdweights":
                    try:
                        key = (
                            repr(inst.ins[0]),
                            str(inst.tile_position),
                            str(inst.perf_mode),
                            str(inst.is_transpose),
                        )
                        synced = inst.sync_info is not None
                    except Exception:
                        key, synced = None, True
                    if key is not None and key == last_key and not synced:
                        removed += 1
                        continue
                    last_key = key
                keep.append(inst)
            if removed:
                insts.clear()
                insts.extend(keep)
                total += removed
    return total


def _get_program():
    if "nc" not in _CACHE:
        _CACHE["nc"] = _build_program()
    return _CACHE["nc"]


def _prep_consts(w1, b1, w2, b2, w3, b3, w4, b4,
                 g1, be1, m1, v1, g2, be2, m2, v2, g3, be3, m3, v3):
    """Host-side folding. Returns dict of device const arrays."""
    f8 = np.float64
    w1 = np.asarray(w1, f8); w2 = np.asarray(w2, f8); w3 = np.asarray(w3, f8)
    w4 = np.asarray(w4, f8)
    b1 = np.asarray(b1, f8); b2 = np.asarray(b2, f8); b3 = np.asarray(b3, f8)
    b4 = np.asarray(b4, f8)

    def fold(g, be, m, v, b):
        s = np.asarray(g, f8) / np.sqrt(np.asarray(v, f8) + EPS)
        c = s * (b - np.asarray(m, f8)) + np.asarray(be, f8)
        return s, c

    s1, c1 = fold(g1, be1, m1, v1, b1)
    s2, c2 = fold(g2, be2, m2, v2, b2)
    s3, c3 = fold(g3, be3, m3, v3, b3)

    W1s = np.sign(w1)  # [384, 81]
    W2s = np.sign(w2)  # [384, 384]
    W3s = np.sign(w3)

    # L1: all input features (u0, {0,1}) -> weights x2
    w1t = (2.0 * W1s).T.astype(BF16)  # [81, 384]

    # Input-feature multiplier for L2/L3: u-form planes ("D") get x2.
    def multv(assign):
        m = np.empty(H, dtype=f8)
        for i, a in enumerate(assign):
            m[i * P : (i + 1) * P] = 2.0 if a == "D" else 1.0
        return m

    mult1 = multv(BIN1)  # forms of L1 outputs = L2 input planes
    mult2 = multv(BIN2)
    W2eff = W2s * mult1[None, :]
    W3eff = W3s * mult2[None, :]

    def pack_lhsT_dr(Weff):
        # DoubleRow packing with overlapping rhs windows: kh=0 reads
        # activation planes (0,1) = features 0..255; kh=1 reads planes
        # (1,2) = features 128..383 with ZERO weights on the repeated
        # plane 1 (i=0), so no pad plane / memset is needed.
        t = np.zeros((P, 6, 2, P), dtype=f8)
        for m in range(3):
            for i in range(2):  # kh=0: features i*128..
                blk = Weff[m * P : (m + 1) * P, i * P : (i + 1) * P]
                t[:, m, i, :] = blk.T
            # kh=1: i=0 stays zero; i=1 = features 256..383
            blk = Weff[m * P : (m + 1) * P, 2 * P : 3 * P]
            t[:, 3 + m, 1, :] = blk.T
        return t.reshape(P, 1536).astype(FP8)

    w2t = pack_lhsT_dr(W2eff)
    w3t = pack_lhsT_dr(W3eff)

    # u-form corrections: sum of signed weights over u-form input features
    def corr(Ws, assign):
        c = np.zeros(H, dtype=f8)
        for i, a in enumerate(assign):
            if a == "D":
                c += Ws[:, i * P : (i + 1) * P].sum(axis=1)
        return c

    # thresholds: u-form: u = [p_raw > phi]; s-form: sign(p_raw - phi)
    phi1 = W1s.sum(axis=1) - c1 / s1          # all L1 inputs u-form
    phi2 = corr(W2s, BIN1) - c2 / s2
    corr3 = corr(W3s, BIN2)

    dw = w4[1] - w4[0]
    db = b4[1] - b4[0]
    scale3 = s3 * dw
    bias3 = dw * (c3 - s3 * corr3)

    aux = np.zeros((P, 28), dtype=f8)
    for m in range(3):
        aux[:, 0 + m] = phi1[m * P : (m + 1) * P]
        aux[:, 3 + m] = -phi1[m * P : (m + 1) * P]
        aux[:, 6 + m] = phi2[m * P : (m + 1) * P]
        aux[:, 9 + m] = -phi2[m * P : (m + 1) * P]
        aux[:, 12 + m] = scale3[m * P : (m + 1) * P]
        aux[:, 15 + m] = bias3[m * P : (m + 1) * P]
        aux[:, 18 + m] = np.abs(dw[m * P : (m + 1) * P])
        aux[:, 21 + m] = -np.abs(dw[m * P : (m + 1) * P])
    aux[:, 24] = db
    aux[:, 25] = -db
    aux = aux.astype(np.float32)

    ones = np.ones((P, 1), dtype=BF16)

    return {"w1t": w1t, "w2t": w2t, "w3t": w3t, "ones": ones, "aux": aux}


def _make_in_maps(inputs):
    x = np.asarray(inputs["x"], np.float32)
    xt = np.ascontiguousarray(x.T.astype(BF16))  # [81, 131072] feature-major
    consts = _prep_consts(
        inputs["w1"], inputs["b1"], inputs["w2"], inputs["b2"],
        inputs["w3"], inputs["b3"], inputs["w4"], inputs["b4"],
        inputs["g1"], inputs["be1"], inputs["m1"], inputs["v1"],
        inputs["g2"], inputs["be2"], inputs["m2"], inputs["v2"],
        inputs["g3"], inputs["be3"], inputs["m3"], inputs["v3"],
    )
    in_maps = []
    for i in range(N_CORES):
        m = {"xt": np.ascontiguousarray(xt[:, i * NB : (i + 1) * NB])}
        m.update(consts)
        in_maps.append(m)
    return in_maps


def kernel(**inputs):
    from concourse.bass_utils import run_bass_kernel_spmd

    nc = _get_program()
    in_maps = _make_in_maps(inputs)
    res = run_bass_kernel_spmd(nc, in_maps, list(range(N_CORES)))
    out = np.concatenate([res.results[i]["out"] for i in range(N_CORES)], axis=0)
    return out.astype(np.float32)


# revision 38
# speedup vs baseline: 1.1874x; 1.1874x over previous
"""Trainium2 Bass kernel for a binarized 4-layer MLP (dense_mlp).

Net (per reference):
  h = sign(x) @ sign(w1).T + b1 ; h = clip(bn1(h), -1, 1)
  h = sign(h) @ sign(w2).T + b2 ; h = clip(bn2(h), -1, 1)
  h = sign(h) @ sign(w3).T + b3 ; h = clip(bn3(h), -1, 1)
  logits = h @ w4.T + b4 ; out = log_softmax(logits)   # 2 classes

Strategy: pure data parallel over 8 cores (batch 131072 -> 8 x 16384).
Host prep: x transposed to feature-major [81, B] and sharded by columns;
sign/BN/bias/log-softmax algebra folded into device weights + thresholds.

On-device per core (feature-major activations, exact bf16 binarized matmuls):
  - u0 = [x > 0] in {0,1} bf16 (GpSimd compare, SBUF->SBUF)
  - sign activations propagate as {0,1} "u-form" (DVE is_gt) or +-1 "s-form"
    (ACT Sign); the 2x / rowsum corrections fold into the next layer's
    weights (+-2, exact in bf16) and per-feature thresholds (host).
  - L3: s3 (BN scale) folds into w3; clip computed as t = min(p+e3, 1) on
    DVE, then r2 = Relu(t+1) = h3+1 on ACT (the -1 folds into the head).
  - head: d = r2 @ dw accumulated in a [8,512] PSUM tile (one row per
    512-col chunk), SBUF->SBUF DMA re-spreads to batch-major [32,128],
    out0 = ln(sigmoid(-(d+db'))), out1 = ln(sigmoid(d+db')).
"""

import os
import sys

import numpy as np

for _p in ("/opt/trn_rl_repo", "/root/.axon_site/_ro/trn_rl_repo"):
    if os.path.isdir(_p) and _p not in sys.path:
        sys.path.insert(0, _p)

import ml_dtypes  # noqa: E402

BF16 = ml_dtypes.bfloat16
FP8 = ml_dtypes.float8_e4m3

# Problem constants (hardcoded per contract)
B_FULL = 131072
N_CORES = 8
NB = B_FULL // N_CORES  # 16384 rows per core
IN = 81
H = 384
EPS = 1e-5
P = 128
WCH = 1024          # free-dim per elementwise tile (2 PSUM banks)
G_NCH = 8           # 512-col chunks per super-chunk
G_COLS = G_NCH * 512  # 4096
N_GROUPS = NB // G_COLS
WPG = G_COLS // WCH
RPT = G_COLS // P  # tail rows per group

_CACHE = {}


def _build_program():
    import concourse.bacc as bacc
    import concourse.bass as bass  # noqa: F401
    import concourse.tile as tile
    from concourse import mybir

    f32 = mybir.dt.float32
    bf16 = mybir.dt.bfloat16
    fp8 = mybir.dt.float8e4
    DR = mybir.MatmulPerfMode.DoubleRow
    AF = mybir.ActivationFunctionType
    ALU = mybir.AluOpType

    nc = bacc.Bacc("TRN2", target_bir_lowering=False, debug=False)

    xt_d = nc.dram_tensor("xt", [IN, NB], bf16, kind="ExternalInput").ap()
    w1t_d = nc.dram_tensor("w1t", [IN, H], bf16, kind="ExternalInput").ap()
    w2t_d = nc.dram_tensor("w2t", [P, 1536], fp8, kind="ExternalInput").ap()
    w3t_d = nc.dram_tensor("w3t", [P, 1536], fp8, kind="ExternalInput").ap()
    dwt_d = nc.dram_tensor("dwt", [P, 3], bf16, kind="ExternalInput").ap()
    aux_d = nc.dram_tensor("aux", [P, 14], f32, kind="ExternalInput").ap()
    out_d = nc.dram_tensor("out", [NB, 2], f32, kind="ExternalOutput").ap()

    with tile.TileContext(nc) as tc:
        with (
            tc.tile_pool(name="consts", bufs=1) as cpool,
            tc.tile_pool(name="xin", bufs=4) as xpool,
            tc.tile_pool(name="u0", bufs=4) as u0pool,
            tc.tile_pool(name="acts", bufs=8) as apool,
            tc.tile_pool(name="tclip", bufs=4) as tpool,
            tc.tile_pool(name="h3", bufs=14) as h3pool,
            tc.tile_pool(name="dsb", bufs=1) as dsbpool,
            tc.tile_pool(name="fin", bufs=2) as fpool,
            tc.tile_pool(name="mm", bufs=3, space="PSUM") as pspool,
            tc.tile_pool(name="mmd", bufs=2, space="PSUM") as psdpool,
            tc.tile_pool(name="dram", bufs=1, space="DRAM") as dpool,
        ):
            dscr = dpool.tile([NB], f32)
            # ---- constants ----
            w1s = cpool.tile([IN, H], bf16)
            nc.sync.dma_start(w1s[:], w1t_d[:])
            w2s = cpool.tile([P, 1536], fp8)
            nc.sync.dma_start(w2s[:], w2t_d[:])
            w3s = cpool.tile([P, 1536], fp8)
            nc.sync.dma_start(w3s[:], w3t_d[:])
            w2r = w2s.rearrange("p (s i c) -> p s i c", i=2, c=P)
            w3r = w3s.rearrange("p (s i c) -> p s i c", i=2, c=P)
            dws = cpool.tile([P, 3], bf16)
            nc.sync.dma_start(dws[:], dwt_d[:])
            aux = cpool.tile([P, 14], f32)
            nc.sync.dma_start(aux[:], aux_d[:])

            def emit_u0(g):
                # stage X: load xT slices (bf16, sign-preserving), compare
                # on DVE (all-bf16 SBUF operands hit the fast 2x/4x modes)
                lst = []
                for w in range(WPG // 2):
                    xf = xpool.tile([IN, 2 * WCH], bf16, tag="xf",
                                    name=f"xf_{g}_{w}")
                    nc.sync.dma_start(
                        xf[:],
                        xt_d[:, g * G_COLS + w * 2 * WCH
                              : g * G_COLS + (w + 1) * 2 * WCH],
                    )
                    u0 = u0pool.tile([IN, 2 * WCH], bf16, tag="u0",
                                     name=f"u0_{g}_{w}")
                    nc.vector.tensor_scalar(u0[:], xf[:], 0.0, None, ALU.is_gt)
                    lst.append(u0)
                return lst

            u0t = emit_u0(0)
            for g in range(N_GROUPS):
                col0 = g * G_COLS
                # prefetch the next group's compares so the PE has L1 work
                # queued up during this group's head tail
                u0t_next = emit_u0(g + 1) if g + 1 < N_GROUPS else None

                # ---- L1 ----
                u1 = []
                for w in range(WPG):
                    ua = apool.tile([P, 3, WCH], fp8, tag="u1")
                    u1.append(ua)
                for m in range(3):
                    for w in range(WPG):
                        ps = pspool.tile([P, WCH], f32, tag="ps")
                        for h in range(2):
                            c0 = w * WCH + h * 512
                            nc.tensor.matmul(
                                ps[:, h * 512 : (h + 1) * 512],
                                w1s[:, m * P : (m + 1) * P],
                                u0t[c0 // (2 * WCH)][
                                    :, c0 % (2 * WCH) : c0 % (2 * WCH) + 512
                                ],
                                start=True,
                                stop=True,
                            )
                        if m < 2:
                            nc.scalar.activation(
                                u1[w][:, m, :], ps[:], AF.Sign,
                                bias=aux[:, m : m + 1], scale=1.0
                            )
                        else:
                            nc.vector.tensor_scalar(
                                u1[w][:, m, :], ps[:], aux[:, 2:3], None,
                                ALU.is_gt
                            )

                # ---- L2 (fp8 DoubleRow, K=512 padded) ----
                u2 = []
                for w in range(WPG):
                    ua = apool.tile([P, 3, WCH], fp8, tag="u2")
                    u2.append(ua)
                for m in range(3):
                    for wp in range(WPG // 2):
                        pss = [
                            pspool.tile([P, WCH], f32, tag="ps", name=f"ps2_{g}_{m}_{wp}_{wi}")
                            for wi in range(2)
                        ]
                        # kh-outer, 4 MMs per weight load
                        for kh in range(2):
                            for wi in range(2):
                                w = wp * 2 + wi
                                for h in range(2):
                                    nc.tensor.matmul(
                                        pss[wi][:, h * 512 : (h + 1) * 512],
                                        w2r[:, kh * 3 + m, :, :],
                                        u1[w][:, kh : kh + 2,
                                              h * 512 : (h + 1) * 512],
                                        start=(kh == 0),
                                        stop=(kh == 1),
                                        perf_mode=DR,
                                    )
                        for wi in range(2):
                            w = wp * 2 + wi
                            if m < 2:
                                nc.scalar.activation(
                                    u2[w][:, m, :], pss[wi][:], AF.Sign,
                                    bias=aux[:, 3 + m : 4 + m], scale=1.0
                                )
                            else:
                                nc.vector.tensor_scalar(
                                    u2[w][:, m, :], pss[wi][:], aux[:, 5:6],
                                    None, ALU.is_gt
                                )

                # ---- L3 (fp8 DoubleRow) + clip ----
                h3 = [[None] * WPG for _ in range(3)]
                for m in range(3):
                    for wp in range(WPG // 2):
                        pss = [
                            pspool.tile([P, WCH], f32, tag="ps", name=f"ps3_{g}_{m}_{wp}_{wi}")
                            for wi in range(2)
                        ]
                        for kh in range(2):
                            for wi in range(2):
                                w = wp * 2 + wi
                                for h in range(2):
                                    nc.tensor.matmul(
                                        pss[wi][:, h * 512 : (h + 1) * 512],
                                        w3r[:, kh * 3 + m, :, :],
                                        u2[w][:, kh : kh + 2,
                                              h * 512 : (h + 1) * 512],
                                        start=(kh == 0),
                                        stop=(kh == 1),
                                        perf_mode=DR,
                                    )
                        for wi in range(2):
                            w = wp * 2 + wi
                            # y3 = s3*p3 + e3 (fp32)
                            t = tpool.tile([P, WCH], f32, tag="t3", name=f"t3_{g}_{m}_{wp}_{wi}")
                            if m < 2:
                                nc.scalar.activation(
                                    t[:], pss[wi][:], AF.Identity,
                                    bias=aux[:, 6 + m : 7 + m],
                                    scale=aux[:, 9 + m : 10 + m],
                                )
                            else:
                                nc.vector.tensor_scalar(
                                    t[:], pss[wi][:],
                                    aux[:, 9 + m : 10 + m], aux[:, 6 + m : 7 + m],
                                    ALU.mult, ALU.add,
                                )
                            # h3 = clip(y3) -> bf16 (DVE)
                            h3c = h3pool.tile([P, WCH], bf16, tag="h3", name=f"h3_{g}_{m}_{wp}_{wi}")
                            nc.vector.tensor_scalar(
                                h3c[:], t[:], 1.0, -1.0, ALU.min, ALU.max
                            )
                            h3[m][w] = h3c

                # ---- head: d per 512-chunk in [1,512] PSUM tiles ----
                dsb = dsbpool.tile([1, G_COLS], f32)
                for r in range(G_NCH):
                    w, h = r // 2, r % 2
                    psd = psdpool.tile([1, 512], f32, tag="psd")
                    for k in range(3):
                        nc.tensor.matmul(
                            psd[:],
                            dws[:, k : k + 1],
                            h3[k][w][:, h * 512 : (h + 1) * 512],
                            start=(k == 0),
                            stop=(k == 2),
                        )
                    dst = dsb[0:1, r * 512 : (r + 1) * 512]
                    if r % 2 == 0:
                        nc.vector.tensor_copy(dst, psd[:])
                    else:
                        nc.scalar.copy(dst, psd[:])

                # re-spread to batch-major [32, 128] via DRAM bounce
                # (direct SBUF->SBUF partition-spread DMA scrambles on HW)
                dsl = dscr[g * G_COLS : (g + 1) * G_COLS]
                nc.sync.dma_start(
                    dsl.rearrange("(one f) -> one f", one=1), dsb[:]
                )
                d2 = fpool.tile([RPT, P], f32, tag="d2")
                nc.sync.dma_start(d2[:], dsl.rearrange("(p j) -> p j", j=P))
                sneg = fpool.tile([RPT, P], f32, tag="sneg")
                nc.scalar.activation(
                    sneg[:], d2[:], AF.Sigmoid, bias=aux[0:RPT, 13:14], scale=-1.0
                )
                spos = fpool.tile([RPT, P], f32, tag="spos")
                nc.scalar.activation(
                    spos[:], d2[:], AF.Sigmoid, bias=aux[0:RPT, 12:13], scale=1.0
                )
                out_t = fpool.tile([RPT, 2 * P], f32, tag="outt")
                ov = out_t.rearrange("p (j c) -> p j c", c=2)
                nc.scalar.activation(ov[:, :, 0], sneg[:], AF.Ln)
                nc.scalar.activation(ov[:, :, 1], spos[:], AF.Ln)
                nc.sync.dma_start(
                    out_d[g * G_COLS : (g + 1) * G_COLS, :].rearrange(
                        "(p j) c -> p (j c)", j=P
                    ),
                    out_t[:],
                )
                u0t = u0t_next

    nc.compile()
    _dedupe_ldweights(nc)
    return nc


def _dedupe_ldweights(nc):
    """Remove InstLdweights whose (weights AP, tile_position, perf_mode,
    is_transpose) matches the previous load on the PE stream (the PE array
    keeps weights until the next load). Only unsynced loads are removed;
    anything carrying semaphore waits/updates refreshes the tracked state
    instead."""
    total = 0
    for f in nc.m.functions:
        for bb in f.blocks:
            insts = bb.instructions
            keep = []
            last_key = None
            removed = 0
            for inst in insts:
                if type(inst).__name__ == "InstLdweights":
                    try:
                        key = (
                            repr(inst.ins[0]),
                            str(inst.tile_position),
                            str(inst.perf_mode),
                            str(inst.is_transpose),
                        )
                        synced = inst.sync_info is not None
                    except Exception:
                        key, synced = None, True
                    if key is not None and key == last_key and not synced:
                        removed += 1
                        continue
                    last_key = key
                keep.append(inst)
            if removed:
                insts.clear()
                insts.extend(keep)
                total += removed
    return total


def _get_program():
    if "nc" not in _CACHE:
        _CACHE["nc"] = _build_program()
    return _CACHE["nc"]


def _prep_consts(w1, b1, w2, b2, w3, b3, w4, b4,
                 g1, be1, m1, v1, g2, be2, m2, v2, g3, be3, m3, v3):
    """Host-side folding. Returns dict of device const arrays."""
    f8 = np.float64
    w1 = np.asarray(w1, f8); w2 = np.asarray(w2, f8); w3 = np.asarray(w3, f8)
    w4 = np.asarray(w4, f8)
    b1 = np.asarray(b1, f8); b2 = np.asarray(b2, f8); b3 = np.asarray(b3, f8)
    b4 = np.asarray(b4, f8)

    def fold(g, be, m, v, b):
        s = np.asarray(g, f8) / np.sqrt(np.asarray(v, f8) + EPS)
        c = s * (b - np.asarray(m, f8)) + np.asarray(be, f8)
        return s, c

    s1, c1 = fold(g1, be1, m1, v1, b1)
    s2, c2 = fold(g2, be2, m2, v2, b2)
    s3, c3 = fold(g3, be3, m3, v3, b3)

    W1s = np.sign(w1)  # [384, 81]
    W2s = np.sign(w2)  # [384, 384]
    W3s = np.sign(w3)

    # L1: all input features (u0) are u-form -> weights x2
    w1t = (2.0 * W1s).T.astype(BF16)  # [81, 384]

    # L2/L3 inputs: m0/m1 chunks (f<256) s-form (+-1), m2 u-form (x2)
    multf = np.where(np.arange(H) < 2 * P, 1.0, 2.0)
    W2eff = W2s * multf[None, :]
    W3eff = W3s * multf[None, :]

    def pack_lhsT_dr(Weff):
        # DoubleRow packing with overlapping rhs windows: kh=0 reads
        # activation planes (0,1) = features 0..255; kh=1 reads planes
        # (1,2) = features 128..383 with ZERO weights on the repeated
        # plane 1 (i=0), so no pad plane / memset is needed.
        t = np.zeros((P, 6, 2, P), dtype=f8)
        for m in range(3):
            for i in range(2):  # kh=0: features i*128..
                blk = Weff[m * P : (m + 1) * P, i * P : (i + 1) * P]
                t[:, m, i, :] = blk.T
            # kh=1: i=0 stays zero; i=1 = features 256..383
            blk = Weff[m * P : (m + 1) * P, 2 * P : 3 * P]
            t[:, 3 + m, 1, :] = blk.T
        return t.reshape(P, 1536).astype(FP8)

    w2t = pack_lhsT_dr(W2eff)
    w3t = pack_lhsT_dr(W3eff)

    # thresholds: u = [p > phi];  s-form ACT: sign(p - phi)
    phi1 = W1s.sum(axis=1) - c1 / s1
    phi2 = W2s[:, 2 * P :].sum(axis=1) - c2 / s2
    # L3: y3 = s3*p3 + e3 with exact +-1/+-2 weights;
    # correction subtracts s3 * sum_{u-form f} W3s[m,f]
    e3 = c3 - s3 * W3s[:, 2 * P :].sum(axis=1)

    dw = w4[1] - w4[0]
    db = b4[1] - b4[0]
    dbp = db  # all h3 chunks stored in clip-form

    dwt = np.zeros((P, 3), dtype=f8)
    for k in range(3):
        dwt[:, k] = dw[k * P : (k + 1) * P]
    dwt = dwt.astype(BF16)

    aux = np.zeros((P, 14), dtype=f8)
    aux[:, 0] = -phi1[0:P]
    aux[:, 1] = -phi1[P : 2 * P]
    aux[:, 2] = phi1[2 * P : 3 * P]
    aux[:, 3] = -phi2[0:P]
    aux[:, 4] = -phi2[P : 2 * P]
    aux[:, 5] = phi2[2 * P : 3 * P]
    for m in range(3):
        aux[:, 6 + m] = e3[m * P : (m + 1) * P]
        aux[:, 9 + m] = s3[m * P : (m + 1) * P]
    aux[:, 12] = dbp
    aux[:, 13] = -dbp
    aux = aux.astype(np.float32)

    return {"w1t": w1t, "w2t": w2t, "w3t": w3t, "dwt": dwt, "aux": aux}


def _make_in_maps(inputs):
    x = np.asarray(inputs["x"], np.float32)
    # bf16 is sign-preserving for randn inputs; halves the x DMA traffic
    xt = np.ascontiguousarray(x.T.astype(BF16))  # [81, 131072] feature-major
    consts = _prep_consts(
        inputs["w1"], inputs["b1"], inputs["w2"], inputs["b2"],
        inputs["w3"], inputs["b3"], inputs["w4"], inputs["b4"],
        inputs["g1"], inputs["be1"], inputs["m1"], inputs["v1"],
        inputs["g2"], inputs["be2"], inputs["m2"], inputs["v2"],
        inputs["g3"], inputs["be3"], inputs["m3"], inputs["v3"],
    )
    in_maps = []
    for i in range(N_CORES):
        m = {"xt": np.ascontiguousarray(xt[:, i * NB : (i + 1) * NB])}
        m.update(consts)
        in_maps.append(m)
    return in_maps


def kernel(**inputs):
    from concourse.bass_utils import run_bass_kernel_spmd

    nc = _get_program()
    in_maps = _make_in_maps(inputs)
    res = run_bass_kernel_spmd(nc, in_maps, list(range(N_CORES)))
    out = np.concatenate([res.results[i]["out"] for i in range(N_CORES)], axis=0)
    return out.astype(np.float32)



# revision 39
# speedup vs baseline: 1.2009x; 1.0113x over previous
"""Trainium2 Bass kernel for a binarized 4-layer MLP (dense_mlp).

Net (per reference):
  h = sign(x) @ sign(w1).T + b1 ; h = clip(bn1(h), -1, 1)
  h = sign(h) @ sign(w2).T + b2 ; h = clip(bn2(h), -1, 1)
  h = sign(h) @ sign(w3).T + b3 ; h = clip(bn3(h), -1, 1)
  logits = h @ w4.T + b4 ; out = log_softmax(logits)   # 2 classes

Strategy: pure data parallel over 8 cores (batch 131072 -> 8 x 16384).
Host prep: x transposed to feature-major [81, B] and sharded by columns;
sign/BN/bias/log-softmax algebra folded into device weights + thresholds.

On-device per core (feature-major activations, exact bf16 binarized matmuls):
  - u0 = [x > 0] in {0,1} bf16 (GpSimd compare, SBUF->SBUF)
  - sign activations propagate as {0,1} "u-form" (DVE is_gt) or +-1 "s-form"
    (ACT Sign); the 2x / rowsum corrections fold into the next layer's
    weights (+-2, exact in bf16) and per-feature thresholds (host).
  - L3: s3 (BN scale) folds into w3; clip computed as t = min(p+e3, 1) on
    DVE, then r2 = Relu(t+1) = h3+1 on ACT (the -1 folds into the head).
  - head: d = r2 @ dw accumulated in a [8,512] PSUM tile (one row per
    512-col chunk), SBUF->SBUF DMA re-spreads to batch-major [32,128],
    out0 = ln(sigmoid(-(d+db'))), out1 = ln(sigmoid(d+db')).
"""

import os
import sys

import numpy as np

for _p in ("/opt/trn_rl_repo", "/root/.axon_site/_ro/trn_rl_repo"):
    if os.path.isdir(_p) and _p not in sys.path:
        sys.path.insert(0, _p)

import ml_dtypes  # noqa: E402

BF16 = ml_dtypes.bfloat16
FP8 = ml_dtypes.float8_e4m3

# Problem constants (hardcoded per contract)
B_FULL = 131072
N_CORES = 8
NB = B_FULL // N_CORES  # 16384 rows per core
IN = 81
H = 384
EPS = 1e-5
P = 128
WCH = 1024          # free-dim per elementwise tile (2 PSUM banks)
G_NCH = 8           # 512-col chunks per super-chunk
G_COLS = G_NCH * 512  # 4096
N_GROUPS = NB // G_COLS
WPG = G_COLS // WCH
RPT = G_COLS // P  # tail rows per group

_CACHE = {}


def _build_program():
    import concourse.bacc as bacc
    import concourse.bass as bass  # noqa: F401
    import concourse.tile as tile
    from concourse import mybir

    f32 = mybir.dt.float32
    bf16 = mybir.dt.bfloat16
    fp8 = mybir.dt.float8e4
    DR = mybir.MatmulPerfMode.DoubleRow
    AF = mybir.ActivationFunctionType
    ALU = mybir.AluOpType

    nc = bacc.Bacc("TRN2", target_bir_lowering=False, debug=False)

    xt_d = nc.dram_tensor("xt", [IN, NB], bf16, kind="ExternalInput").ap()
    w1t_d = nc.dram_tensor("w1t", [IN, H], bf16, kind="ExternalInput").ap()
    w2t_d = nc.dram_tensor("w2t", [P, 1536], fp8, kind="ExternalInput").ap()
    w3t_d = nc.dram_tensor("w3t", [P, 1536], fp8, kind="ExternalInput").ap()
    dwt_d = nc.dram_tensor("dwt", [P, 3], bf16, kind="ExternalInput").ap()
    aux_d = nc.dram_tensor("aux", [P, 14], f32, kind="ExternalInput").ap()
    out_d = nc.dram_tensor("out", [NB, 2], f32, kind="ExternalOutput").ap()

    with tile.TileContext(nc) as tc:
        with (
            tc.tile_pool(name="consts", bufs=1) as cpool,
            tc.tile_pool(name="xin", bufs=4) as xpool,
            tc.tile_pool(name="u0", bufs=4) as u0pool,
            tc.tile_pool(name="acts", bufs=8) as apool,
            tc.tile_pool(name="tclip", bufs=4) as tpool,
            tc.tile_pool(name="h3", bufs=14) as h3pool,
            tc.tile_pool(name="dsb", bufs=1) as dsbpool,
            tc.tile_pool(name="fin", bufs=2) as fpool,
            tc.tile_pool(name="mm", bufs=3, space="PSUM") as pspool,
            tc.tile_pool(name="mmd", bufs=2, space="PSUM") as psdpool,
            tc.tile_pool(name="dram", bufs=1, space="DRAM") as dpool,
        ):
            dscr = dpool.tile([NB], f32)
            # ---- constants ----
            w1s = cpool.tile([IN, H], bf16)
            nc.sync.dma_start(w1s[:], w1t_d[:])
            w2s = cpool.tile([P, 1536], fp8)
            nc.sync.dma_start(w2s[:], w2t_d[:])
            w3s = cpool.tile([P, 1536], fp8)
            nc.sync.dma_start(w3s[:], w3t_d[:])
            w2r = w2s.rearrange("p (s i c) -> p s i c", i=2, c=P)
            w3r = w3s.rearrange("p (s i c) -> p s i c", i=2, c=P)
            dws = cpool.tile([P, 3], bf16)
            nc.sync.dma_start(dws[:], dwt_d[:])
            aux = cpool.tile([P, 14], f32)
            nc.sync.dma_start(aux[:], aux_d[:])

            def emit_u0(g):
                # stage X: load xT slices (bf16, sign-preserving), compare
                # on DVE (all-bf16 SBUF operands hit the fast 2x/4x modes)
                lst = []
                for w in range(WPG // 2):
                    xf = xpool.tile([IN, 2 * WCH], bf16, tag="xf",
                                    name=f"xf_{g}_{w}")
                    nc.sync.dma_start(
                        xf[:],
                        xt_d[:, g * G_COLS + w * 2 * WCH
                              : g * G_COLS + (w + 1) * 2 * WCH],
                    )
                    u0 = u0pool.tile([IN, 2 * WCH], bf16, tag="u0",
                                     name=f"u0_{g}_{w}")
                    nc.vector.tensor_scalar(u0[:], xf[:], 0.0, None, ALU.is_gt)
                    lst.append(u0)
                return lst

            u0t = emit_u0(0)
            for g in range(N_GROUPS):
                col0 = g * G_COLS
                # prefetch the next group's compares so the PE has L1 work
                # queued up during this group's head tail
                u0t_next = emit_u0(g + 1) if g + 1 < N_GROUPS else None

                # ---- L1 ----
                u1 = []
                for w in range(WPG):
                    ua = apool.tile([P, 3, WCH], fp8, tag="u1")
                    u1.append(ua)
                for m in range(3):
                    for w in range(WPG):
                        ps = pspool.tile([P, WCH], f32, tag="ps")
                        for h in range(2):
                            c0 = w * WCH + h * 512
                            nc.tensor.matmul(
                                ps[:, h * 512 : (h + 1) * 512],
                                w1s[:, m * P : (m + 1) * P],
                                u0t[c0 // (2 * WCH)][
                                    :, c0 % (2 * WCH) : c0 % (2 * WCH) + 512
                                ],
                                start=True,
                                stop=True,
                            )
                        if m < 2:
                            nc.scalar.activation(
                                u1[w][:, m, :], ps[:], AF.Sign,
                                bias=aux[:, m : m + 1], scale=1.0
                            )
                        else:
                            nc.vector.tensor_scalar(
                                u1[w][:, m, :], ps[:], aux[:, 2:3], None,
                                ALU.is_gt
                            )

                # ---- L2 (fp8 DoubleRow, K=512 padded) ----
                u2 = []
                for w in range(WPG):
                    ua = apool.tile([P, 3, WCH], fp8, tag="u2")
                    u2.append(ua)
                for m in range(3):
                    for wp in range(WPG // 2):
                        pss = [
                            pspool.tile([P, WCH], f32, tag="ps", name=f"ps2_{g}_{m}_{wp}_{wi}")
                            for wi in range(2)
                        ]
                        # kh-outer, 4 MMs per weight load
                        for kh in range(2):
                            for wi in range(2):
                                w = wp * 2 + wi
                                for h in range(2):
                                    nc.tensor.matmul(
                                        pss[wi][:, h * 512 : (h + 1) * 512],
                                        w2r[:, kh * 3 + m, :, :],
                                        u1[w][:, kh : kh + 2,
                                              h * 512 : (h + 1) * 512],
                                        start=(kh == 0),
                                        stop=(kh == 1),
                                        perf_mode=DR,
                                    )
                        for wi in range(2):
                            w = wp * 2 + wi
                            if m < 2:
                                nc.scalar.activation(
                                    u2[w][:, m, :], pss[wi][:], AF.Sign,
                                    bias=aux[:, 3 + m : 4 + m], scale=1.0
                                )
                            else:
                                nc.vector.tensor_scalar(
                                    u2[w][:, m, :], pss[wi][:], aux[:, 5:6],
                                    None, ALU.is_gt
                                )

                # ---- L3 (fp8 DoubleRow) + clip ----
                h3 = [[None] * WPG for _ in range(3)]
                for m in range(3):
                    for wp in range(WPG // 2):
                        pss = [
                            pspool.tile([P, WCH], f32, tag="ps", name=f"ps3_{g}_{m}_{wp}_{wi}")
                            for wi in range(2)
                        ]
                        for kh in range(2):
                            for wi in range(2):
                                w = wp * 2 + wi
                                for h in range(2):
                                    nc.tensor.matmul(
                                        pss[wi][:, h * 512 : (h + 1) * 512],
                                        w3r[:, kh * 3 + m, :, :],
                                        u2[w][:, kh : kh + 2,
                                              h * 512 : (h + 1) * 512],
                                        start=(kh == 0),
                                        stop=(kh == 1),
                                        perf_mode=DR,
                                    )
                        for wi in range(2):
                            w = wp * 2 + wi
                            # y3 = s3*p3 + e3 (fp32)
                            t = tpool.tile([P, WCH], f32, tag="t3", name=f"t3_{g}_{m}_{wp}_{wi}")
                            # affine is form-free: alternate m=0 between ACT
                            # and DVE to balance the two drain engines
                            if m == 1 or (m == 0 and (g + wp) % 2 == 0):
                                nc.scalar.activation(
                                    t[:], pss[wi][:], AF.Identity,
                                    bias=aux[:, 6 + m : 7 + m],
                                    scale=aux[:, 9 + m : 10 + m],
                                )
                            else:
                                nc.vector.tensor_scalar(
                                    t[:], pss[wi][:],
                                    aux[:, 9 + m : 10 + m], aux[:, 6 + m : 7 + m],
                                    ALU.mult, ALU.add,
                                )
                            # h3 = clip(y3) -> bf16 (DVE)
                            h3c = h3pool.tile([P, WCH], bf16, tag="h3", name=f"h3_{g}_{m}_{wp}_{wi}")
                            nc.vector.tensor_scalar(
                                h3c[:], t[:], 1.0, -1.0, ALU.min, ALU.max
                            )
                            h3[m][w] = h3c

                # ---- head: d per 512-chunk in [1,512] PSUM tiles ----
                dsb = dsbpool.tile([1, G_COLS], f32)
                for r in range(G_NCH):
                    w, h = r // 2, r % 2
                    psd = psdpool.tile([1, 512], f32, tag="psd")
                    for k in range(3):
                        nc.tensor.matmul(
                            psd[:],
                            dws[:, k : k + 1],
                            h3[k][w][:, h * 512 : (h + 1) * 512],
                            start=(k == 0),
                            stop=(k == 2),
                        )
                    dst = dsb[0:1, r * 512 : (r + 1) * 512]
                    if r % 2 == 0:
                        nc.vector.tensor_copy(dst, psd[:])
                    else:
                        nc.scalar.copy(dst, psd[:])

                # re-spread to batch-major [32, 128] via DRAM bounce
                # (direct SBUF->SBUF partition-spread DMA scrambles on HW)
                dsl = dscr[g * G_COLS : (g + 1) * G_COLS]
                nc.sync.dma_start(
                    dsl.rearrange("(one f) -> one f", one=1), dsb[:]
                )
                d2 = fpool.tile([RPT, P], f32, tag="d2")
                nc.sync.dma_start(d2[:], dsl.rearrange("(p j) -> p j", j=P))
                sneg = fpool.tile([RPT, P], f32, tag="sneg")
                nc.scalar.activation(
                    sneg[:], d2[:], AF.Sigmoid, bias=aux[0:RPT, 13:14], scale=-1.0
                )
                spos = fpool.tile([RPT, P], f32, tag="spos")
                nc.scalar.activation(
                    spos[:], d2[:], AF.Sigmoid, bias=aux[0:RPT, 12:13], scale=1.0
                )
                out_t = fpool.tile([RPT, 2 * P], f32, tag="outt")
                ov = out_t.rearrange("p (j c) -> p j c", c=2)
                nc.scalar.activation(ov[:, :, 0], sneg[:], AF.Ln)
                nc.scalar.activation(ov[:, :, 1], spos[:], AF.Ln)
                nc.sync.dma_start(
                    out_d[g * G_COLS : (g + 1) * G_COLS, :].rearrange(
                        "(p j) c -> p (j c)", j=P
                    ),
                    out_t[:],
                )
                u0t = u0t_next

    nc.compile()
    _dedupe_ldweights(nc)
    return nc


def _dedupe_ldweights(nc):
    """Remove InstLdweights whose (weights AP, tile_position, perf_mode,
    is_transpose) matches the previous load on the PE stream (the PE array
    keeps weights until the next load). Only unsynced loads are removed;
    anything carrying semaphore waits/updates refreshes the tracked state
    instead."""
    total = 0
    for f in nc.m.functions:
        for bb in f.blocks:
            insts = bb.instructions
            keep = []
            last_key = None
            removed = 0
            for inst in insts:
                if type(inst).__name__ == "InstLdweights":
                    try:
                        key = (
                            repr(inst.ins[0]),
                            str(inst.tile_position),
                            str(inst.perf_mode),
                            str(inst.is_transpose),
                        )
                        synced = inst.sync_info is not None
                    except Exception:
                        key, synced = None, True
                    if key is not None and key == last_key and not synced:
                        removed += 1
                        continue
                    last_key = key
                keep.append(inst)
            if removed:
                insts.clear()
                insts.extend(keep)
                total += removed
    return total


def _get_program():
    if "nc" not in _CACHE:
        _CACHE["nc"] = _build_program()
    return _CACHE["nc"]


def _prep_consts(w1, b1, w2, b2, w3, b3, w4, b4,
                 g1, be1, m1, v1, g2, be2, m2, v2, g3, be3, m3, v3):
    """Host-side folding. Returns dict of device const arrays."""
    f8 = np.float64
    w1 = np.asarray(w1, f8); w2 = np.asarray(w2, f8); w3 = np.asarray(w3, f8)
    w4 = np.asarray(w4, f8)
    b1 = np.asarray(b1, f8); b2 = np.asarray(b2, f8); b3 = np.asarray(b3, f8)
    b4 = np.asarray(b4, f8)

    def fold(g, be, m, v, b):
        s = np.asarray(g, f8) / np.sqrt(np.asarray(v, f8) + EPS)
        c = s * (b - np.asarray(m, f8)) + np.asarray(be, f8)
        return s, c

    s1, c1 = fold(g1, be1, m1, v1, b1)
    s2, c2 = fold(g2, be2, m2, v2, b2)
    s3, c3 = fold(g3, be3, m3, v3, b3)

    W1s = np.sign(w1)  # [384, 81]
    W2s = np.sign(w2)  # [384, 384]
    W3s = np.sign(w3)

    # L1: all input features (u0) are u-form -> weights x2
    w1t = (2.0 * W1s).T.astype(BF16)  # [81, 384]

    # L2/L3 inputs: m0/m1 chunks (f<256) s-form (+-1), m2 u-form (x2)
    multf = np.where(np.arange(H) < 2 * P, 1.0, 2.0)
    W2eff = W2s * multf[None, :]
    W3eff = W3s * multf[None, :]

    def pack_lhsT_dr(Weff):
        # DoubleRow packing with overlapping rhs windows: kh=0 reads
        # activation planes (0,1) = features 0..255; kh=1 reads planes
        # (1,2) = features 128..383 with ZERO weights on the repeated
        # plane 1 (i=0), so no pad plane / memset is needed.
        t = np.zeros((P, 6, 2, P), dtype=f8)
        for m in range(3):
            for i in range(2):  # kh=0: features i*128..
                blk = Weff[m * P : (m + 1) * P, i * P : (i + 1) * P]
                t[:, m, i, :] = blk.T
            # kh=1: i=0 stays zero; i=1 = features 256..383
            blk = Weff[m * P : (m + 1) * P, 2 * P : 3 * P]
            t[:, 3 + m, 1, :] = blk.T
        return t.reshape(P, 1536).astype(FP8)

    w2t = pack_lhsT_dr(W2eff)
    w3t = pack_lhsT_dr(W3eff)

    # thresholds: u = [p > phi];  s-form ACT: sign(p - phi)
    phi1 = W1s.sum(axis=1) - c1 / s1
    phi2 = W2s[:, 2 * P :].sum(axis=1) - c2 / s2
    # L3: y3 = s3*p3 + e3 with exact +-1/+-2 weights;
    # correction subtracts s3 * sum_{u-form f} W3s[m,f]
    e3 = c3 - s3 * W3s[:, 2 * P :].sum(axis=1)

    dw = w4[1] - w4[0]
    db = b4[1] - b4[0]
    dbp = db  # all h3 chunks stored in clip-form

    dwt = np.zeros((P, 3), dtype=f8)
    for k in range(3):
        dwt[:, k] = dw[k * P : (k + 1) * P]
    dwt = dwt.astype(BF16)

    aux = np.zeros((P, 14), dtype=f8)
    aux[:, 0] = -phi1[0:P]
    aux[:, 1] = -phi1[P : 2 * P]
    aux[:, 2] = phi1[2 * P : 3 * P]
    aux[:, 3] = -phi2[0:P]
    aux[:, 4] = -phi2[P : 2 * P]
    aux[:, 5] = phi2[2 * P : 3 * P]
    for m in range(3):
        aux[:, 6 + m] = e3[m * P : (m + 1) * P]
        aux[:, 9 + m] = s3[m * P : (m + 1) * P]
    aux[:, 12] = dbp
    aux[:, 13] = -dbp
    aux = aux.astype(np.float32)

    return {"w1t": w1t, "w2t": w2t, "w3t": w3t, "dwt": dwt, "aux": aux}


def _make_in_maps(inputs):
    x = np.asarray(inputs["x"], np.float32)
    # bf16 is sign-preserving for randn inputs; halves the x DMA traffic
    xt = np.ascontiguousarray(x.T.astype(BF16))  # [81, 131072] feature-major
    consts = _prep_consts(
        inputs["w1"], inputs["b1"], inputs["w2"], inputs["b2"],
        inputs["w3"], inputs["b3"], inputs["w4"], inputs["b4"],
        inputs["g1"], inputs["be1"], inputs["m1"], inputs["v1"],
        inputs["g2"], inputs["be2"], inputs["m2"], inputs["v2"],
        inputs["g3"], inputs["be3"], inputs["m3"], inputs["v3"],
    )
    in_maps = []
    for i in range(N_CORES):
        m = {"xt": np.ascontiguousarray(xt[:, i * NB : (i + 1) * NB])}
        m.update(consts)
        in_maps.append(m)
    return in_maps


def kernel(**inputs):
    from concourse.bass_utils import run_bass_kernel_spmd

    nc = _get_program()
    in_maps = _make_in_maps(inputs)
    res = run_bass_kernel_spmd(nc, in_maps, list(range(N_CORES)))
    out = np.concatenate([res.results[i]["out"] for i in range(N_CORES)], axis=0)
    return out.astype(np.float32)

